# revision 5
# baseline (speedup 1.0000x reference)
"""Trainium2 Bass kernel for ConvolutionalAutoregressiveModel sampling.

Incremental AR decoding (equivalent to the reference's per-step full
recompute, by causality) with cached conv activations.

Per-core layout (data-parallel over 8 cores, BL=4 batch each):
 - ST state tile [32, 416]: batch on partitions (rows 0..3); col 3(t+7)+c
   holds component c of conv-input tap t: (s[t], loc[t-1], tmp[t-1]),
   where eps[t-1] = loc[t-1] + tmp[t-1] enters layer 1 through duplicated
   weight rows. s[] is prefilled host-side; loc/tmp are written per step
   as [4,1] partition-0-aligned columns (engine APs must be 32-aligned
   in partition start).
 - per step, one DVE 32x32 block transpose converts ST cols [3i, 3i+32)
   into the layer-1 window [taps-on-partitions, batch]; rows 24..31 are
   garbage multiplied by zero weight rows.
 - z1/z2 histories pair-stacked into [128, *] stores (two time steps per
   partition dim, A/B parity copies) so conv layers 2/3 are 4 K=128
   matmuls; 3 of 4 prestart off the critical path.
 - head matmul swapped: out = z3.T @ [loc_w ls_w] -> psum [BL, 2], so
   sampling runs batch-on-partitions and writes ST directly.
"""

import numpy as np

B, T, H, KT = 32, 128, 64, 8
NCORES = 8
BL = B // NCORES
STC = 3 * (T + KT) + 8          # 416 state columns
LOG_SQRT_2PI = 0.9189385332046727


def _host_noise():
    """noise[t] = random.normal(split(key(42), T)[t], (B,1)).

    Plain jax on the DEFAULT device: the random stream is platform
    dependent and must match what reference() produces in this env.
    """
    import jax
    from jax import random
    import jax.numpy as jnp

    keys = random.split(jax.random.key(42), T)
    n = np.stack([np.asarray(random.normal(k, (B, 1), jnp.float32))[:, 0]
                  for k in keys])  # [T, B]
    n = n.astype(np.float32)
    n = np.where(np.abs(n) < np.float32(1e-30), np.float32(1e-30), n)
    return n


def _build_program():
    from concourse import bacc, tile, mybir

    dt = mybir.dt.float32
    AF = mybir.ActivationFunctionType

    nc = bacc.Bacc(
        "TRN2", target_bir_lowering=False, debug=False, num_devices=NCORES)

    d_w0 = nc.dram_tensor("w0fix", (32, H), dt, kind="ExternalInput")
    d_w2 = nc.dram_tensor("w2pair", (2 * H, 4 * H), dt, kind="ExternalInput")
    d_w3 = nc.dram_tensor("w3pair", (2 * H, 4 * H), dt, kind="ExternalInput")
    d_heads = nc.dram_tensor("headsw", (H, 2), dt, kind="ExternalInput")
    d_convb = nc.dram_tensor("convb", (H, 3), dt, kind="ExternalInput")
    d_headb = nc.dram_tensor("headbp", (BL, 2), dt, kind="ExternalInput")
    d_noise = nc.dram_tensor("noisebp", (BL, T), dt, kind="ExternalInput")
    d_stini = nc.dram_tensor("stinit", (32, STC), dt, kind="ExternalInput")
    d_out = nc.dram_tensor("st_out", (32, STC), dt, kind="ExternalOutput")

    NHC = (T + KT - 2) // 2 + 2  # 68 pair-columns in z histories

    with tile.TileContext(nc) as tc:
        with (
            tc.tile_pool(name="const", bufs=1) as cpool,
            tc.tile_pool(name="state", bufs=1) as spool,
            tc.tile_pool(name="work", bufs=2) as wpool,
            tc.tile_pool(name="psum", bufs=2, space="PSUM") as ppool,
        ):
            w0 = cpool.tile([32, H], dt)
            w2 = cpool.tile([2 * H, 4 * H], dt)
            w3 = cpool.tile([2 * H, 4 * H], dt)
            heads = cpool.tile([H, 2], dt)
            convb = cpool.tile([H, 3], dt)
            headb = cpool.tile([BL, 2], dt)
            noise = cpool.tile([BL, T], dt)

            st = spool.tile([32, STC], dt)
            z1s = [spool.tile([2 * H, NHC * BL], dt, name=f"z1s{p}") for p in range(2)]
            z2s = [spool.tile([2 * H, NHC * BL], dt, name=f"z2s{p}") for p in range(2)]

            nc.sync.dma_start(w0[:], d_w0[:])
            nc.sync.dma_start(w2[:], d_w2[:])
            nc.sync.dma_start(w3[:], d_w3[:])
            nc.sync.dma_start(heads[:], d_heads[:])
            nc.sync.dma_start(convb[:], d_convb[:])
            nc.sync.dma_start(headb[:], d_headb[:])
            nc.sync.dma_start(noise[:], d_noise[:])
            nc.sync.dma_start(st[:], d_stini[:])

            for z in (*z1s, *z2s):
                nc.gpsimd.memset(z[:], 0.0)

            def hwrite_ap(zpair, t):
                """(A_ap, B_ap) write positions for step-t output."""
                uA = t + KT - 1
                uB = uA + 1
                a = zpair[0][(uA % 2) * H:(uA % 2) * H + H,
                             (uA // 2) * BL:(uA // 2) * BL + BL]
                b = zpair[1][(uB % 2) * H:(uB % 2) * H + H,
                             (uB // 2) * BL:(uB // 2) * BL + BL]
                return a, b

            def win_ap(zpair, t, j):
                """rhs [128, BL] for pair j of the window ending at step t."""
                store = t % 2
                c0 = t // 2 if store == 0 else (t + 1) // 2
                z = zpair[store]
                return z[:, (c0 + j) * BL:(c0 + j + 1) * BL]

            psum2 = {}
            psum3 = {}

            def prestart(layer_psum, zpair, w, tag, i):
                pt = ppool.tile([H, BL], dt, name=f"{tag}_{i}", tag=tag)
                layer_psum[i] = pt
                for j in range(3):
                    nc.tensor.matmul(pt[:], w[:, j * H:(j + 1) * H],
                                     win_ap(zpair, i, j),
                                     start=(j == 0), stop=False)
                return pt

            prestart(psum2, z1s, w2, "psum2", 0)
            prestart(psum3, z2s, w3, "psum3", 0)

            for i in range(T):
                # --- L1 window: 32x32 block transpose of ST cols [3i,3i+32) ---
                ring = wpool.tile([32, 32], dt, name=f"ring_{i}", tag="ring")
                nc.vector.transpose(ring[:], st[0:32, 3 * i:3 * i + 32])

                psum1 = ppool.tile([H, BL], dt, name=f"psum1_{i}", tag="psum1")
                nc.tensor.matmul(psum1[:], w0[:], ring[0:32, 0:BL],
                                 start=True, stop=True)

                # --- relu1 -> z1 stores (critical parity first) ---
                aA, aB = hwrite_ap(z1s, i)
                crit, other = (aA, aB) if (i + 1) % 2 == 0 else (aB, aA)
                nc.scalar.activation(crit, psum1[:], AF.Relu, bias=convb[:, 0:1])
                nc.gpsimd.tensor_copy(other, crit)

                # --- L2 final pair, then prestart step i+1 ---
                p2 = psum2[i]
                nc.tensor.matmul(p2[:], w2[:, 3 * H:4 * H], win_ap(z1s, i, 3),
                                 start=False, stop=True)
                if i + 1 < T:
                    prestart(psum2, z1s, w2, "psum2", i + 1)

                aA, aB = hwrite_ap(z2s, i)
                crit, other = (aA, aB) if (i + 1) % 2 == 0 else (aB, aA)
                nc.scalar.activation(crit, p2[:], AF.Relu, bias=convb[:, 1:2])
                nc.gpsimd.tensor_copy(other, crit)

                # --- L3 ---
                p3 = psum3[i]
                nc.tensor.matmul(p3[:], w3[:, 3 * H:4 * H], win_ap(z2s, i, 3),
                                 start=False, stop=True)
                if i + 1 < T:
                    prestart(psum3, z2s, w3, "psum3", i + 1)

                z3cur = wpool.tile([H, BL], dt, name=f"z3_{i}", tag="z3")
                nc.scalar.activation(z3cur[:], p3[:], AF.Relu, bias=convb[:, 2:3])

                # --- swapped head: psumh [BL, 2] = z3.T @ (loc_w | ls_w) ---
                psumh = ppool.tile([BL, 2], dt, name=f"psumh_{i}", tag="psumh")
                nc.tensor.matmul(psumh[:], z3cur[:], heads[:], start=True, stop=True)

                # --- sampling (batch on partitions, all writes at part 0) ---
                expls = wpool.tile([BL, 1], dt, name=f"expls_{i}", tag="expls")
                nc.scalar.activation(expls[:], psumh[:, 1:2], AF.Exp,
                                     bias=headb[:, 1:2])
                nc.scalar.activation(st[0:BL, 3 * i + 25:3 * i + 26],
                                     psumh[:, 0:1], AF.Identity,
                                     bias=headb[:, 0:1])
                nc.vector.tensor_mul(st[0:BL, 3 * i + 26:3 * i + 27],
                                     expls[:], noise[:, i:i + 1])

            nc.sync.dma_start(d_out[:], st[:])

    nc.compile()
    return nc


_PROG_CACHE = {}


def _get_program():
    if "nc" not in _PROG_CACHE:
        _PROG_CACHE["nc"] = _build_program()
    return _PROG_CACHE["nc"]


def _pack_inputs(s, conv_ws, conv_bs, loc_w, loc_b, ls_w, ls_b, noise):
    s = np.asarray(s, np.float32)
    W0, W1, W2 = [np.asarray(w, np.float32) for w in conv_ws]
    b0, b1, b2 = [np.asarray(b, np.float32) for b in conv_bs]
    loc_w = np.asarray(loc_w, np.float32)
    ls_w = np.asarray(ls_w, np.float32)
    loc_b = np.asarray(loc_b, np.float32)
    ls_b = np.asarray(ls_b, np.float32)

    w0fix = np.zeros((32, H), np.float32)
    for j in range(KT):
        w0fix[3 * j + 0] = W0[j][0]
        w0fix[3 * j + 1] = W0[j][1]
        w0fix[3 * j + 2] = W0[j][1]

    def pack_pairs(W):
        out = np.zeros((2 * H, 4 * H), np.float32)
        for j in range(4):
            out[0:H, j * H:(j + 1) * H] = W[2 * j]
            out[H:2 * H, j * H:(j + 1) * H] = W[2 * j + 1]
        return out

    w2 = pack_pairs(W1)
    w3 = pack_pairs(W2)
    headsw = np.concatenate([loc_w, ls_w], axis=1)          # [64, 2]
    convb = np.stack([b0, b1, b2], axis=1)                  # [64, 3]
    headb = np.tile(np.array([[loc_b[0], ls_b[0]]], np.float32), (BL, 1))

    in_maps = []
    for c in range(NCORES):
        sb = s[c * BL:(c + 1) * BL, :]                      # [BL, T]
        nb = noise[:, c * BL:(c + 1) * BL].T.copy()         # [BL, T]
        stinit = np.zeros((32, STC), np.float32)
        for t in range(T):
            stinit[0:BL, 3 * t + 21] = sb[:, t]
        in_maps.append({
            "w0fix": w0fix, "w2pair": w2, "w3pair": w3,
            "headsw": headsw, "convb": convb, "headbp": headb,
            "noisebp": nb, "stinit": stinit,
        })
    return in_maps


def kernel(s, x, conv_ws, conv_bs, loc_w, loc_b, ls_w, ls_b, _trace=False):
    from concourse import bass_utils

    noise = _host_noise()                                   # [T, B]
    nc = _get_program()
    in_maps = _pack_inputs(s, conv_ws, conv_bs, loc_w, loc_b, ls_w, ls_b, noise)

    res = bass_utils.run_bass_kernel_spmd(
        nc, in_maps, core_ids=list(range(NCORES)), trace=_trace)

    locs = np.zeros((T, B), np.float32)
    tmps = np.zeros((T, B), np.float32)
    tidx = np.arange(T)
    for c in range(NCORES):
        h = res.results[c]["st_out"]                        # [32, STC]
        locs[:, c * BL:(c + 1) * BL] = h[0:BL, 3 * tidx + 25].T
        tmps[:, c * BL:(c + 1) * BL] = h[0:BL, 3 * tidx + 26].T

    pred = (locs + tmps).T                                  # [B, T] f32
    n64 = noise.astype(np.float64)
    ls_est = np.log(np.abs(tmps.astype(np.float64))) - np.log(np.abs(n64))
    logp = (-0.5 * n64 ** 2 - ls_est - LOG_SQRT_2PI).T.astype(np.float32)

    out = (logp, pred)
    if _trace:
        return out, res
    return out
